# revision 3
# baseline (speedup 1.0000x reference)
"""Jitter-gather kernel for Trainium2 (8 NeuronCores, data parallel).

out[:, :, t] = quantized[:, :, idx[t]], idx[t] in {t-1, t, t+1} derived on host
from the tiny [T] random vectors.

The kernel is HBM-bandwidth bound (~358 GB/s per core). Mandatory traffic is
the 32 MiB f32 input read; the output is stored as bf16 (16 MiB instead of
32 MiB), well inside the 2e-2 relative-error budget (bf16 rounding is ~4e-3),
cutting total traffic to ~49 MiB per core. DVE (the only engine with
copy_predicated) runs two full predicated passes per tile, ~8.6 us, just
under the ~8.8 us/tile DMA cadence, so the pipeline is jointly DMA/DVE-bound.

Engine layout:
  - SP  (HWDGE): 16 tile loads (f32) plus the [128, 2T] u8 mask load (the
        masks ship pre-replicated across partitions; at u8 that is only
        1 MiB of extra HBM traffic and removes the on-device replication
        chain from the DVE critical path).
  - ACT: per-tile full copy ot = cast_bf16(xt), and the bf16 stores on its
        HWDGE ring.
  - DVE: two copy_predicated passes per tile (u8 mask, f32 data, bf16 out),
        with a drain between them (their masked byte-writes are disjoint but
        share SBUF cachelines) and a drain before each dve_sem increment.

Tiles 0, 1, 2 and 15 are processed in 4 column-chunks of 1024 so the
DVE pipeline starts earlier (ramp) and the final store shrinks (tail).
A chunk's pred-right covers out columns [c0-1, c1-1) (its data needs column
c1 which is only loaded with the next chunk), so the last-tile store of
chunk k waits for the preds of chunk k+1.
"""

from contextlib import ExitStack

import numpy as np

from concourse import bass, mybir
from concourse.bass_utils import run_bass_kernel_spmd

B, C, T = 32, 512, 4096
PROB_PERCENT = 12
N_CORES = 8
P = 128
ROWS_PER_CORE = (B // N_CORES) * C  # 2048
N_TILES = ROWS_PER_CORE // P  # 16
NBUF = 8
CH = 4  # column-chunks for the first/last tile
CW = T // CH

_CACHE: dict = {}


def _units():
    """Work units: (tile, c0, c1). Ramp/tail tiles are column-chunked."""
    units = []
    for i in range(N_TILES):
        if i in (0, 1, 2, N_TILES - 1):
            for k in range(CH):
                units.append((i, k * CW, (k + 1) * CW))
        else:
            units.append((i, 0, T))
    return units


def _build_nc() -> bass.Bass:
    f32 = mybir.dt.float32
    bf16 = mybir.dt.bfloat16
    u8 = mybir.dt.uint8
    nc = bass.Bass()
    x = nc.declare_dram_parameter("x", [ROWS_PER_CORE, T], f32, isOutput=False)
    m2 = nc.declare_dram_parameter("m2", [P, 2 * T], u8, isOutput=False)
    y = nc.declare_dram_parameter("y", [ROWS_PER_CORE, T], bf16, isOutput=True)

    units = _units()
    NU = len(units)
    # A slot semaphore may only ever have ONE outstanding DMA: the 16
    # per-SDMA-engine increments of back-to-back transfers interleave, so a
    # wait for a partial count can pass while the earlier transfer is still
    # in flight (HW-verified corruption). Chunked tiles therefore use one
    # sem per (slot, chunk index).
    exp_load = [0] * NU  # load sem value to wait for after this unit's load
    sem_key = [None] * NU  # (slot, chunk_idx) of this unit's load sem
    uses = {}
    first_unit_of_tile = {}
    last_unit_of_tile = {}
    for j, (i, c0, c1) in enumerate(units):
        s = i % NBUF
        k = j - first_unit_of_tile[i] if i in first_unit_of_tile else 0
        first_unit_of_tile.setdefault(i, j)
        last_unit_of_tile[i] = j
        key = (s, k)
        uses[key] = uses.get(key, 0) + 1
        sem_key[j] = key
        exp_load[j] = 16 * uses[key]
    dve_after_tile = {i: last_unit_of_tile[i] + 1 for i in range(N_TILES)}
    # store units: (tile, c0, c1, dve_sem gate)
    store_units = []
    for i in range(N_TILES):
        if i == N_TILES - 1:
            for k in range(CH):
                gate = first_unit_of_tile[i] + min(k + 1, CH - 1) + 1
                store_units.append((i, k * CW, (k + 1) * CW, gate))
        else:
            store_units.append((i, 0, T, dve_after_tile[i]))
    # store_sems[slot] value after all stores of tile i have completed
    stores_thru_tile = {}
    run = [0] * NBUF
    for i in range(N_TILES):
        run[i % NBUF] += sum(1 for (ti, _, _, _) in store_units if ti == i)
        stores_thru_tile[i] = 16 * run[i % NBUF]

    ctx = ExitStack()
    with ctx:
        m2_s = ctx.enter_context(nc.sbuf_tensor("m2_s", [P, 2 * T], u8))
        xts = [
            ctx.enter_context(nc.sbuf_tensor(f"xt{b}", [P, T], f32))
            for b in range(NBUF)
        ]
        ots = [
            ctx.enter_context(nc.sbuf_tensor(f"ot{b}", [P, T], bf16))
            for b in range(NBUF)
        ]
        ml_s = m2_s[:, 0:T]
        mr_s = m2_s[:, T : 2 * T]
        # [128, 2, T] views of the mask pair for chunk-pair DMA loads
        m2v = m2_s[:].rearrange("p (h t) -> p h t", h=2)
        m2d = m2[:].rearrange("p (h t) -> p h t", h=2)

        sems = ExitStack()
        with sems, nc.Block(no_gpsimd_drain=True) as block:
            mask_sems = [
                sems.enter_context(nc.semaphore(f"mask_sem{k}")) for k in range(CH)
            ]
            copy_sem = sems.enter_context(nc.semaphore("copy_sem"))
            dve_sem = sems.enter_context(nc.semaphore("dve_sem"))
            load_sems = {
                key: sems.enter_context(nc.semaphore(f"load_sem{key[0]}_{key[1]}"))
                for key in uses
            }
            store_sems = [
                sems.enter_context(nc.semaphore(f"store_sem{b}")) for b in range(NBUF)
            ]

            @block.sync
            def _(sync: bass.BassEngine):
                for j, (i, c0, c1) in enumerate(units):
                    s = i % NBUF
                    if i >= NBUF and j == first_unit_of_tile[i]:
                        # xt[s] last read by the preds of tile i-NBUF
                        sync.wait_ge(dve_sem, dve_after_tile[i - NBUF])
                    rows = slice(i * P, (i + 1) * P)
                    sync.dma_start(
                        out=xts[s][:, c0:c1], in_=x[rows, c0:c1]
                    ).then_inc(load_sems[sem_key[j]], 16)
                    if j < CH:
                        # Mask chunk-pair k (ml and mr columns [k*CW,(k+1)*CW))
                        # interleaves with the first tile's chunk loads, so
                        # the first preds are unblocked ~2 us earlier than a
                        # monolithic 1 MiB mask load would allow.
                        k = j
                        cs = slice(k * CW, (k + 1) * CW)
                        sync.dma_start(
                            out=m2v[:, :, cs], in_=m2d[:, :, cs]
                        ).then_inc(mask_sems[k], 16)

            @block.vector
            def _(vector: bass.BassVectorEngine):
                for j, (i, c0, c1) in enumerate(units):
                    s = i % NBUF
                    vector.wait_ge(copy_sem, j + 1)
                    if j < CH:
                        vector.wait_ge(mask_sems[j], 16)
                    xt, ot = xts[s], ots[s]
                    # Two disjoint masked passes (ml-true and mr-true never
                    # overlap); both read only xt -> no drain between them.
                    # Edge rules give ml[0] == 0 and mr[T-1] == 0, so the
                    # shifted views need no halo.
                    a = c0 - 1 if c0 > 0 else 0
                    vector.copy_predicated(
                        ot[:, a : c1 - 1], mr_s[:, a : c1 - 1], xt[:, a + 1 : c1]
                    )
                    # The masked byte-writes of the two passes are disjoint but
                    # share 16 B SBUF cachelines; partial-line RMWs of two
                    # in-flight instructions can lose bytes. Drain between.
                    vector.drain()
                    b_ = max(c0, 1)
                    vector.copy_predicated(
                        ot[:, b_:c1], ml_s[:, b_:c1], xt[:, b_ - 1 : c1 - 1]
                    )
                    # drain before signalling: sem updates must not outrun the
                    # engine's SBUF writes (HW-verified failure mode without it)
                    vector.drain().then_inc(dve_sem, 1)

            @block.scalar
            def _(scalar: bass.BassScalarEngine):
                # Per-unit full copy ot = cast_bf16(xt); stores interleaved:
                # the store of tile i-1 goes right after the first copy-unit
                # of tile i; the chunked last-tile stores go at the end.
                store_iter = iter(
                    [su for su in store_units if su[0] < N_TILES - 1]
                )
                for j, (i, c0, c1) in enumerate(units):
                    s = i % NBUF
                    scalar.wait_ge(load_sems[sem_key[j]], exp_load[j])
                    if i >= NBUF and j == first_unit_of_tile[i]:
                        # ot[s] last read by the stores of tile i-NBUF
                        scalar.wait_ge(store_sems[s], stores_thru_tile[i - NBUF])
                    scalar.copy(ots[s][:, c0:c1], xts[s][:, c0:c1])
                    scalar.drain().then_inc(copy_sem, 1)
                    if i >= 1 and j == first_unit_of_tile[i]:
                        si, sc0, sc1, gate = next(store_iter)
                        ssl = si % NBUF
                        scalar.wait_ge(dve_sem, gate)
                        rows = slice(si * P, (si + 1) * P)
                        scalar.dma_start(
                            out=y[rows, sc0:sc1], in_=ots[ssl][:, sc0:sc1]
                        ).then_inc(store_sems[ssl], 16)
                # last tile's chunked stores
                for si, sc0, sc1, gate in store_units:
                    if si < N_TILES - 1:
                        continue
                    ssl = si % NBUF
                    scalar.wait_ge(dve_sem, gate)
                    rows = slice(si * P, (si + 1) * P)
                    scalar.dma_start(
                        out=y[rows, sc0:sc1], in_=ots[ssl][:, sc0:sc1]
                    ).then_inc(store_sems[ssl], 16)
                # drain: all stores landed before the program ends
                for s in range(NBUF):
                    n = sum(1 for (ti, _, _, _) in store_units if ti % NBUF == s)
                    scalar.wait_ge(store_sems[s], 16 * n)

    return nc


def _masks(replace_rand: np.ndarray, dir_rand: np.ndarray):
    t = np.arange(T)
    direction = np.where(dir_rand == 0, -1, 1)
    neighbor = t + direction
    neighbor = np.where(t == 0, 1, neighbor)
    neighbor = np.where(t == T - 1, T - 2, neighbor)
    replace = replace_rand < PROB_PERCENT
    idx = np.where(replace, neighbor, t)
    d = idx - t
    m2 = np.empty((P, 2 * T), dtype=np.uint8)
    m2[:, :T] = (d == -1).astype(np.uint8)[None, :]
    m2[:, T:] = (d == 1).astype(np.uint8)[None, :]
    return m2


def kernel(quantized: np.ndarray, replace_rand: np.ndarray, dir_rand: np.ndarray):
    quantized = np.asarray(quantized, dtype=np.float32)
    replace_rand = np.asarray(replace_rand)
    dir_rand = np.asarray(dir_rand)

    if "nc" not in _CACHE:
        _CACHE["nc"] = _build_nc()
    nc = _CACHE["nc"]

    m2 = _masks(replace_rand, dir_rand)
    shards = quantized.reshape(N_CORES, ROWS_PER_CORE, T)
    in_maps = [
        {"x": np.ascontiguousarray(shards[i]), "m2": m2} for i in range(N_CORES)
    ]
    res = run_bass_kernel_spmd(nc, in_maps, list(range(N_CORES)))
    out = np.concatenate(
        [np.asarray(r["y"]).astype(np.float32)[None] for r in res.results], axis=0
    )
    return out.reshape(B, C, T)


# revision 4
# speedup vs baseline: 1.0025x; 1.0025x over previous
"""Jitter-gather kernel for Trainium2 (8 NeuronCores, data parallel).

out[:, :, t] = quantized[:, :, idx[t]], idx[t] in {t-1, t, t+1} derived on host
from the tiny [T] random vectors.

The kernel is HBM-bandwidth bound (~358 GB/s per core). Mandatory traffic is
the 32 MiB f32 input read; the output is stored as bf16 (16 MiB instead of
32 MiB), well inside the 2e-2 relative-error budget (bf16 rounding is ~4e-3),
cutting total traffic to ~49 MiB per core. DVE (the only engine with
copy_predicated) runs two full predicated passes per tile, ~8.6 us, just
under the ~8.8 us/tile DMA cadence, so the pipeline is jointly DMA/DVE-bound.

Engine layout:
  - SP  (HWDGE): 16 tile loads (f32) plus the [128, 2T] u8 mask load (the
        masks ship pre-replicated across partitions; at u8 that is only
        1 MiB of extra HBM traffic and removes the on-device replication
        chain from the DVE critical path).
  - ACT: per-tile full copy ot = cast_bf16(xt), and the bf16 stores on its
        HWDGE ring.
  - DVE: two copy_predicated passes per tile (u8 mask, f32 data, bf16 out),
        with a drain between them (their masked byte-writes are disjoint but
        share SBUF cachelines) and a drain before each dve_sem increment.

Tiles 0, 1, 2 and 15 are processed in 4 column-chunks of 1024 so the
DVE pipeline starts earlier (ramp) and the final store shrinks (tail).
A chunk's pred-right covers out columns [c0-1, c1-1) (its data needs column
c1 which is only loaded with the next chunk), so the last-tile store of
chunk k waits for the preds of chunk k+1.
"""

from contextlib import ExitStack

import numpy as np

from concourse import bass, mybir
from concourse.bass_utils import run_bass_kernel_spmd

B, C, T = 32, 512, 4096
PROB_PERCENT = 12
N_CORES = 8
P = 128
ROWS_PER_CORE = (B // N_CORES) * C  # 2048
N_TILES = ROWS_PER_CORE // P  # 16
NBUF = 8
CH = 4  # mask is loaded in CH chunk-pairs of CW columns
CW = T // CH

# Per-tile column-chunk boundaries. Tile 0 ramps in 512-col steps so the
# DVE pipeline starts ~1.5 us earlier; tiles 1-2 bridge the ramp; the last
# tile tapers so the final (post-DVE) store is small.
_TILE_CHUNKS = {
    0: [0, 512, 1024, 2048, 3072, 4096],
    1: [0, 1024, 2048, 3072, 4096],
    2: [0, 1024, 2048, 3072, 4096],
    N_TILES - 1: [0, 1024, 2048, 3072, 3584, 4096],
}

_CACHE: dict = {}


def _units():
    """Work units: (tile, c0, c1). Ramp/tail tiles are column-chunked."""
    units = []
    for i in range(N_TILES):
        bounds = _TILE_CHUNKS.get(i, [0, T])
        for c0, c1 in zip(bounds[:-1], bounds[1:]):
            units.append((i, c0, c1))
    return units


def _build_nc() -> bass.Bass:
    f32 = mybir.dt.float32
    bf16 = mybir.dt.bfloat16
    u8 = mybir.dt.uint8
    nc = bass.Bass()
    x = nc.declare_dram_parameter("x", [ROWS_PER_CORE, T], f32, isOutput=False)
    m2 = nc.declare_dram_parameter("m2", [P, 2 * T], u8, isOutput=False)
    y = nc.declare_dram_parameter("y", [ROWS_PER_CORE, T], bf16, isOutput=True)

    units = _units()
    NU = len(units)
    # A slot semaphore may only ever have ONE outstanding DMA: the 16
    # per-SDMA-engine increments of back-to-back transfers interleave, so a
    # wait for a partial count can pass while the earlier transfer is still
    # in flight (HW-verified corruption). Chunked tiles therefore use one
    # sem per (slot, chunk index).
    exp_load = [0] * NU  # load sem value to wait for after this unit's load
    sem_key = [None] * NU  # (slot, chunk_idx) of this unit's load sem
    uses = {}
    first_unit_of_tile = {}
    last_unit_of_tile = {}
    for j, (i, c0, c1) in enumerate(units):
        s = i % NBUF
        k = j - first_unit_of_tile[i] if i in first_unit_of_tile else 0
        first_unit_of_tile.setdefault(i, j)
        last_unit_of_tile[i] = j
        key = (s, k)
        uses[key] = uses.get(key, 0) + 1
        sem_key[j] = key
        exp_load[j] = 16 * uses[key]
    dve_after_tile = {i: last_unit_of_tile[i] + 1 for i in range(N_TILES)}
    # store units: (tile, c0, c1, dve_sem gate). A chunk's pred-right writes
    # its range's last column from the NEXT chunk's pred pass, so the store
    # of chunk k waits for the preds of chunk k+1.
    store_units = []
    last = N_TILES - 1
    lb = _TILE_CHUNKS[last]
    nlc = len(lb) - 1
    for i in range(N_TILES):
        if i == last:
            for k in range(nlc):
                gate = first_unit_of_tile[i] + min(k + 1, nlc - 1) + 1
                store_units.append((i, lb[k], lb[k + 1], gate))
        else:
            store_units.append((i, 0, T, dve_after_tile[i]))
    # store_sems[slot] value after all stores of tile i have completed
    stores_thru_tile = {}
    run = [0] * NBUF
    for i in range(N_TILES):
        run[i % NBUF] += sum(1 for (ti, _, _, _) in store_units if ti == i)
        stores_thru_tile[i] = 16 * run[i % NBUF]

    ctx = ExitStack()
    with ctx:
        m2_s = ctx.enter_context(nc.sbuf_tensor("m2_s", [P, 2 * T], u8))
        xts = [
            ctx.enter_context(nc.sbuf_tensor(f"xt{b}", [P, T], f32))
            for b in range(NBUF)
        ]
        ots = [
            ctx.enter_context(nc.sbuf_tensor(f"ot{b}", [P, T], bf16))
            for b in range(NBUF)
        ]
        ml_s = m2_s[:, 0:T]
        mr_s = m2_s[:, T : 2 * T]
        # [128, 2, T] views of the mask pair for chunk-pair DMA loads
        m2v = m2_s[:].rearrange("p (h t) -> p h t", h=2)
        m2d = m2[:].rearrange("p (h t) -> p h t", h=2)

        sems = ExitStack()
        with sems, nc.Block(no_gpsimd_drain=True) as block:
            mask_sems = [
                sems.enter_context(nc.semaphore(f"mask_sem{k}")) for k in range(CH)
            ]
            copy_sem = sems.enter_context(nc.semaphore("copy_sem"))
            dve_sem = sems.enter_context(nc.semaphore("dve_sem"))
            load_sems = {
                key: sems.enter_context(nc.semaphore(f"load_sem{key[0]}_{key[1]}"))
                for key in uses
            }
            store_sems = [
                sems.enter_context(nc.semaphore(f"store_sem{b}")) for b in range(NBUF)
            ]

            @block.sync
            def _(sync: bass.BassEngine):
                for j, (i, c0, c1) in enumerate(units):
                    s = i % NBUF
                    if i >= NBUF and j == first_unit_of_tile[i]:
                        # xt[s] last read by the preds of tile i-NBUF
                        sync.wait_ge(dve_sem, dve_after_tile[i - NBUF])
                    rows = slice(i * P, (i + 1) * P)
                    sync.dma_start(
                        out=xts[s][:, c0:c1], in_=x[rows, c0:c1]
                    ).then_inc(load_sems[sem_key[j]], 16)
                    if i == 0 and c0 % CW == 0:
                        # Mask chunk-pair k (ml and mr columns [k*CW,(k+1)*CW))
                        # interleaves with the first tile's chunk loads, so
                        # the first preds are unblocked much earlier than a
                        # monolithic 1 MiB mask load would allow.
                        k = c0 // CW
                        cs = slice(k * CW, (k + 1) * CW)
                        sync.dma_start(
                            out=m2v[:, :, cs], in_=m2d[:, :, cs]
                        ).then_inc(mask_sems[k], 16)

            @block.vector
            def _(vector: bass.BassVectorEngine):
                for j, (i, c0, c1) in enumerate(units):
                    s = i % NBUF
                    vector.wait_ge(copy_sem, j + 1)
                    if i == 0:
                        m = (c1 - 1) // CW
                        if m > (c0 - 1) // CW or c0 == 0:
                            vector.wait_ge(mask_sems[m], 16)
                    xt, ot = xts[s], ots[s]
                    # Two disjoint masked passes (ml-true and mr-true never
                    # overlap); both read only xt -> no drain between them.
                    # Edge rules give ml[0] == 0 and mr[T-1] == 0, so the
                    # shifted views need no halo.
                    a = c0 - 1 if c0 > 0 else 0
                    vector.copy_predicated(
                        ot[:, a : c1 - 1], mr_s[:, a : c1 - 1], xt[:, a + 1 : c1]
                    )
                    # The masked byte-writes of the two passes are disjoint but
                    # share 16 B SBUF cachelines; partial-line RMWs of two
                    # in-flight instructions can lose bytes. Drain between.
                    vector.drain()
                    b_ = max(c0, 1)
                    vector.copy_predicated(
                        ot[:, b_:c1], ml_s[:, b_:c1], xt[:, b_ - 1 : c1 - 1]
                    )
                    # drain before signalling: sem updates must not outrun the
                    # engine's SBUF writes (HW-verified failure mode without it)
                    vector.drain().then_inc(dve_sem, 1)

            @block.scalar
            def _(scalar: bass.BassScalarEngine):
                # Per-unit full copy ot = cast_bf16(xt); stores interleaved:
                # the store of tile i-1 goes right after the first copy-unit
                # of tile i; the chunked last-tile stores go at the end.
                store_iter = iter(
                    [su for su in store_units if su[0] < N_TILES - 1]
                )
                for j, (i, c0, c1) in enumerate(units):
                    s = i % NBUF
                    scalar.wait_ge(load_sems[sem_key[j]], exp_load[j])
                    if i >= NBUF and j == first_unit_of_tile[i]:
                        # ot[s] last read by the stores of tile i-NBUF
                        scalar.wait_ge(store_sems[s], stores_thru_tile[i - NBUF])
                    scalar.copy(ots[s][:, c0:c1], xts[s][:, c0:c1])
                    scalar.drain().then_inc(copy_sem, 1)
                    if i >= 1 and j == first_unit_of_tile[i]:
                        si, sc0, sc1, gate = next(store_iter)
                        ssl = si % NBUF
                        scalar.wait_ge(dve_sem, gate)
                        rows = slice(si * P, (si + 1) * P)
                        scalar.dma_start(
                            out=y[rows, sc0:sc1], in_=ots[ssl][:, sc0:sc1]
                        ).then_inc(store_sems[ssl], 16)
                # last tile's chunked stores
                for si, sc0, sc1, gate in store_units:
                    if si < N_TILES - 1:
                        continue
                    ssl = si % NBUF
                    scalar.wait_ge(dve_sem, gate)
                    rows = slice(si * P, (si + 1) * P)
                    scalar.dma_start(
                        out=y[rows, sc0:sc1], in_=ots[ssl][:, sc0:sc1]
                    ).then_inc(store_sems[ssl], 16)
                # drain: all stores landed before the program ends
                for s in range(NBUF):
                    n = sum(1 for (ti, _, _, _) in store_units if ti % NBUF == s)
                    scalar.wait_ge(store_sems[s], 16 * n)

    return nc


def _masks(replace_rand: np.ndarray, dir_rand: np.ndarray):
    t = np.arange(T)
    direction = np.where(dir_rand == 0, -1, 1)
    neighbor = t + direction
    neighbor = np.where(t == 0, 1, neighbor)
    neighbor = np.where(t == T - 1, T - 2, neighbor)
    replace = replace_rand < PROB_PERCENT
    idx = np.where(replace, neighbor, t)
    d = idx - t
    m2 = np.empty((P, 2 * T), dtype=np.uint8)
    m2[:, :T] = (d == -1).astype(np.uint8)[None, :]
    m2[:, T:] = (d == 1).astype(np.uint8)[None, :]
    return m2


def kernel(quantized: np.ndarray, replace_rand: np.ndarray, dir_rand: np.ndarray):
    quantized = np.asarray(quantized, dtype=np.float32)
    replace_rand = np.asarray(replace_rand)
    dir_rand = np.asarray(dir_rand)

    if "nc" not in _CACHE:
        _CACHE["nc"] = _build_nc()
    nc = _CACHE["nc"]

    m2 = _masks(replace_rand, dir_rand)
    shards = quantized.reshape(N_CORES, ROWS_PER_CORE, T)
    in_maps = [
        {"x": np.ascontiguousarray(shards[i]), "m2": m2} for i in range(N_CORES)
    ]
    res = run_bass_kernel_spmd(nc, in_maps, list(range(N_CORES)))
    out = np.concatenate(
        [np.asarray(r["y"]).astype(np.float32)[None] for r in res.results], axis=0
    )
    return out.reshape(B, C, T)
